# revision 13
# baseline (speedup 1.0000x reference)
"""NegSNR loss on TRN2: two-stage rfft (16384 = 128x128) as matmuls.

Per core (128 samples): stage 1 computes Y[b,d] = sum_a x[128a+b] W128^{ad}
(x as stationary, [C|S] moving). Stage 2 folds the twiddle W_N^{bd} and the
outer DFT into per-d constant matrices: X[128c+d] = sum_b Y[b,d] M_d[b,c],
computed only for the in-band bins k in [364, 2276) (c in [2,18)), with
out-of-band columns zeroed. P1 = Xr^2 + Xi^2 returned per core; the final
gather/log/mean runs on host.

Sync constraint: the BIR/walrus path cannot split >1 semaphore wait on a
Matmult (fused ldweights). Tiny 1x1 "absorber" matmuls soak up each
cross-engine dependency so every real matmul carries at most one wait.
"""

import numpy as np

import concourse.bass as bass
import concourse.mybir as mybir
import concourse.tile as tile
from concourse.bass_utils import run_bass_kernel_spmd

B = 1024
N = 16384
NCORES = 8
SPC = B // NCORES  # samples per core = 128
FS = 30.0
F = N // 2 + 1
MIN_IDX = 364
MAX_IDX = 2276
DENOM = float(MAX_IDX - MIN_IDX - 3)  # 1909
C0, NC16, ND = 2, 16, 128  # c in [2,18), d in [0,128)
CHUNK = 8
NCH = SPC // CHUNK  # 16

_MM_DT = mybir.dt.float32r


def _build_weights():
    a = np.arange(128)
    d = np.arange(128)
    ang1 = 2.0 * np.pi * np.outer(a, d) / 128.0
    cs = np.concatenate([np.cos(ang1), -np.sin(ang1)], axis=1) / 128.0  # [a, 256]

    b = np.arange(128)
    cgrid = np.arange(C0, C0 + NC16)
    k = 128 * cgrid[None, :] + d[:, None]  # [d, c]
    ang2 = 2.0 * np.pi * b[None, :, None] * k[:, None, :] / float(N)  # [d, b, c]
    mr = np.cos(ang2) / 128.0
    mi = -np.sin(ang2) / 128.0
    oob = ((k < MIN_IDX) | (k >= MAX_IDX))[:, None, :]  # [d, 1, c]
    mr = np.where(oob, 0.0, mr)
    mi = np.where(oob, 0.0, mi)
    # m1[b, d*32 + (0:16)] = Mr_d ; m1[b, d*32 + (16:32)] = Mi_d
    m1 = np.concatenate([mr, mi], axis=2)  # [d, b, 32]
    m2 = np.concatenate([-mi, mr], axis=2)
    m1 = np.ascontiguousarray(m1.transpose(1, 0, 2)).reshape(128, ND * 32)
    m2 = np.ascontiguousarray(m2.transpose(1, 0, 2)).reshape(128, ND * 32)
    return (
        cs.astype(np.float32),
        m1.astype(np.float32),
        m2.astype(np.float32),
    )


def _ref_indices(t_hz):
    # exact argmin over f_i = 15*i/8192 with first-min tie-break
    u = t_hz.astype(np.float64) * 8192.0 / 15.0
    lo = np.floor(u).astype(np.int64)
    f = 15.0 / 8192.0
    d_lo = np.abs(lo * f - u * f)
    d_hi = np.abs((lo + 1) * f - u * f)
    return np.where(d_lo <= d_hi, lo, lo + 1)


def _strip_implied_waits(nc):
    """Drop sync waits that are transitively implied by other waits.

    The BIR/walrus backend allows only 1 sync wait on Matmult and DMACopy
    descriptors. Tile emits conservative wait sets (e.g. a PSUM WAW wait on
    PE plus a WAR wait on the drain engine, where the drain itself already
    waited on that PE value). A wait (S>=v) is droppable when another kept
    wait (S'>=v') has a producer whose happens-before closure already
    guarantees S>=v. Each DMA HW queue is modeled as its own FIFO engine.
    """
    from collections import defaultdict

    insts = [i for blk in nc.m.functions[0].blocks for i in blk.instructions]

    timeline = defaultdict(list)  # sem id -> [(cum_value, inst_index)]
    upd_sem = {}
    for idx, inst in enumerate(insts):
        si = inst.sync_info
        if not si:
            continue
        for u in si.on_update:
            if u.update_value is None:
                continue
            prev = timeline[u.id][-1][0] if timeline[u.id] else 0
            timeline[u.id].append((prev + u.update_value, idx))
            if "DMAHW" in (u.ant_name or "") or idx not in upd_sem:
                upd_sem[idx] = u.id

    def producer(sem_id, val):
        for cum, idx in timeline[sem_id]:
            if cum >= val:
                return idx
        return None

    def veng(idx):
        inst = insts[idx]
        if isinstance(inst, mybir.InstDMACopy) and idx in upd_sem:
            return ("q", upd_sem[idx])
        return ("e", str(inst.engine))

    # closure[idx]: sem id -> guaranteed min value at completion of insts[idx]
    closure = {}
    running = defaultdict(dict)  # veng -> sem floor dict
    for idx, inst in enumerate(insts):
        g = dict(running[veng(idx)])
        si = inst.sync_info
        if si:
            for w in si.on_wait:
                if w.wait_value is None:
                    continue
                g[w.id] = max(g.get(w.id, 0), w.wait_value)
                p = producer(w.id, w.wait_value)
                if p is not None and p < idx and p in closure:
                    for s, v in closure[p].items():
                        g[s] = max(g.get(s, 0), v)
            for u in si.on_update:
                if u.update_value is None:
                    continue
                cum = next(c for c, i2 in timeline[u.id] if i2 == idx)
                g[u.id] = max(g.get(u.id, 0), cum)
        closure[idx] = g
        running[veng(idx)] = g

    limits = {mybir.InstMatmult: 1, mybir.InstDMACopy: 1}
    for idx, inst in enumerate(insts):
        si = inst.sync_info
        if not si or len(si.on_wait) <= 1:
            continue
        kept = list(si.on_wait)
        changed = True
        while changed and len(kept) > 1:
            changed = False
            for w in kept:
                if w.wait_value is None:
                    continue
                implied = False
                for w2 in kept:
                    if w2 is w or w2.wait_value is None:
                        continue
                    p = producer(w2.id, w2.wait_value)
                    if (
                        p is not None
                        and p < idx
                        and closure.get(p, {}).get(w.id, 0) >= w.wait_value
                    ):
                        implied = True
                        break
                if implied:
                    kept.remove(w)
                    changed = True
                    break
        if len(kept) < len(si.on_wait):
            si.on_wait = kept
        lim = limits.get(type(inst))
        assert lim is None or len(kept) <= lim, (
            f"{type(inst).__name__} {inst.name} has {len(kept)} waits: "
            f"{[(w.ant_name, w.wait_value) for w in kept]}"
        )


def _build_program():
    nc = bass.Bass("TRN2", target_bir_lowering=False, debug=False, num_devices=NCORES)
    f32 = mybir.dt.float32

    x_d = nc.dram_tensor("x", [SPC, N], _MM_DT, kind="ExternalInput").ap()
    cs_d = nc.dram_tensor("cs", [128, 256], _MM_DT, kind="ExternalInput").ap()
    m1_d = nc.dram_tensor("m1", [128, ND * 32], _MM_DT, kind="ExternalInput").ap()
    m2_d = nc.dram_tensor("m2", [128, ND * 32], _MM_DT, kind="ExternalInput").ap()
    p1_d = nc.dram_tensor("p1", [SPC, ND * NC16], f32, kind="ExternalOutput").ap()

    with tile.TileContext(nc) as tc:
        with (
            tc.tile_pool(name="const", bufs=1) as const,
            tc.tile_pool(name="xin", bufs=3) as xin,
            tc.tile_pool(name="big", bufs=1) as big,
            tc.tile_pool(name="ps", bufs=2, space="PSUM") as ps,
        ):
            cs_sb = const.tile([128, 256], _MM_DT)
            nc.sync.dma_start(out=cs_sb, in_=cs_d)
            m1_sb = const.tile([128, ND, 32], _MM_DT)
            nc.sync.dma_start(out=m1_sb, in_=m1_d)
            m2_sb = const.tile([128, ND, 32], _MM_DT)
            nc.sync.dma_start(out=m2_sb, in_=m2_d)

            y_sb = big.tile([128, SPC, 256], _MM_DT)  # [b, s, Yr|Yi]
            sq = big.tile([SPC, ND, 32], f32)  # [s, d, Xr^2|Xi^2]
            p1_sb = big.tile([SPC, ND, NC16], f32)

            def dummy_mm(out_cell, lhs_cell, rhs_cell):
                # N=1 matmuls fail the walrus ISA check; use [1,1]x[1,2]
                nc.tensor.matmul(
                    out=out_cell, lhsT=lhs_cell, rhs=rhs_cell, start=True, stop=True
                )

            # stage 1: per chunk of 8 samples, 8 matmuls into a 4-bank PSUM
            # tile, one big drain alternating ACT/DVE.
            yc0 = ps.tile([128, CHUNK, 256], f32, tag="ps")
            # warmup absorbers: PE waits once on each const DMA
            dummy_mm(yc0[0:1, 0, 0:2], cs_sb[0:1, 0:1], cs_sb[0:1, 0:2])
            dummy_mm(yc0[0:1, 0, 0:2], m1_sb[0:1, 0, 0:1], m1_sb[0:1, 0, 0:2])
            dummy_mm(yc0[0:1, 0, 0:2], m2_sb[0:1, 0, 0:1], m2_sb[0:1, 0, 0:2])

            for c in range(NCH):
                yc = yc0 if c == 0 else ps.tile([128, CHUNK, 256], f32, tag="ps")
                xt = xin.tile([128, CHUNK, 128], _MM_DT)  # [a, s, b]
                nc.sync.dma_start(
                    out=xt,
                    in_=x_d[c * CHUNK : (c + 1) * CHUNK, :].rearrange(
                        "s (a b) -> a s b", b=128
                    ),
                )
                if c >= 2:
                    # absorb the WAR wait on chunk c-2's drain engine
                    dummy_mm(yc[0:1, 0, 0:2], cs_sb[0:1, 0:1], cs_sb[0:1, 0:2])
                for si in range(CHUNK):
                    nc.tensor.matmul(
                        out=yc[:, si, :],
                        lhsT=xt[:, si, :],
                        rhs=cs_sb,
                        start=True,
                        stop=True,
                    )
                dst = y_sb[:, c * CHUNK : (c + 1) * CHUNK, :]
                if c % 2 == 0:
                    nc.scalar.copy(out=dst, in_=yc)
                else:
                    nc.vector.tensor_copy(out=dst, in_=yc)

            # stage 2
            xp0 = ps.tile([SPC, 16, 32], f32, tag="ps")
            # absorb y_sb readiness: last ACT drain (chunk 14) and last DVE
            # drain (chunk 15)
            dummy_mm(xp0[0:1, 0, 0:2], y_sb[0:1, 14 * CHUNK, 0:1], y_sb[0:1, 14 * CHUNK, 0:2])
            dummy_mm(xp0[0:1, 0, 0:2], y_sb[0:1, 15 * CHUNK, 0:1], y_sb[0:1, 15 * CHUNK, 0:2])
            for db in range(8):
                xp = xp0 if db == 0 else ps.tile([SPC, 16, 32], f32, tag="ps")
                for dd in range(16):
                    d = db * 16 + dd
                    nc.tensor.matmul(
                        out=xp[:, dd, :],
                        lhsT=y_sb[:, :, d],
                        rhs=m1_sb[:, d, :],
                        start=True,
                        stop=False,
                    )
                    nc.tensor.matmul(
                        out=xp[:, dd, :],
                        lhsT=y_sb[:, :, 128 + d],
                        rhs=m2_sb[:, d, :],
                        start=False,
                        stop=True,
                    )
                nc.scalar.activation(
                    out=sq[:, db * 16 : (db + 1) * 16, :],
                    in_=xp,
                    func=mybir.ActivationFunctionType.Square,
                )

            nc.vector.tensor_add(
                out=p1_sb, in0=sq[:, :, 0:16], in1=sq[:, :, 16:32]
            )
            nc.sync.dma_start(out=p1_d, in_=p1_sb)

    _strip_implied_waits(nc)
    return nc


_CACHE = {}


def _get_program():
    if "nc" not in _CACHE:
        _CACHE["nc"] = _build_program()
        _CACHE["w"] = _build_weights()
    return _CACHE["nc"], _CACHE["w"]


def _run(outputs, targets, trace=False):
    nc, (cs, m1, m2) = _get_program()
    outputs = np.ascontiguousarray(np.asarray(outputs, dtype=np.float32))
    targets = np.asarray(targets, dtype=np.float32)

    in_maps = [
        {"x": outputs[i * SPC : (i + 1) * SPC], "cs": cs, "m1": m1, "m2": m2}
        for i in range(NCORES)
    ]
    res = run_bass_kernel_spmd(nc, in_maps, list(range(NCORES)), trace=trace)
    p1 = np.concatenate([res.results[i]["p1"] for i in range(NCORES)], axis=0)

    t_hz = targets[:, 0].astype(np.float64) / 60.0
    ref = _ref_indices(t_hz)

    def jidx(k):
        return (k % 128) * NC16 + (k // 128 - C0)

    p1 = p1.astype(np.float64)
    band = p1.sum(axis=1)
    sidx = np.arange(B)
    excl = (
        p1[sidx, jidx(ref - 1)] + p1[sidx, jidx(ref)] + p1[sidx, jidx(ref + 1)]
    )
    pulse = p1[sidx, jidx(ref)]
    other = (band - excl) / DENOM
    snr = 10.0 * np.log10(pulse / other)
    loss = -np.mean(snr)
    return np.float32(loss), res.exec_time_ns


def kernel(**inputs):
    loss, _ = _run(inputs["outputs"], inputs["targets"], trace=False)
    return np.asarray(loss, dtype=np.float32)


# revision 14
# speedup vs baseline: 1.9431x; 1.9431x over previous
"""NegSNR loss on TRN2: two-stage rfft (16384 = 128x128) as fp16 matmuls.

Per core (128 samples): stage 1 computes Y[b,d] = sum_a x[128a+b] W128^{ad}
for d in [0,64] only (x real => Y_{128-d} = conj(Y_d)): 65 cos cols + 63 sin
cols = 128 columns. Stage 2 folds the twiddle W_N^{bd} and the outer DFT
into per-d constant matrices; conjugate pairs (d, 128-d) share the same
stationary Y columns, so each of the 64 pairs needs just two matmuls of 64
moving columns: X over in-band bins k in [364, 2276) (c in [2,18)), with
out-of-band columns zeroed. P1 = Xr^2 + Xi^2 is returned per core; the
final gather/log/mean runs on host.

Sync constraint: the BIR/walrus path cannot split >1 semaphore wait on a
Matmult (fused ldweights) or DMACopy. Tiny [1x1]x[1x2] "absorber" matmuls
soak up each cross-engine dependency so every real matmul carries at most
one wait, and _strip_implied_waits drops waits already implied transitively.
"""

import numpy as np

import concourse.bass as bass
import concourse.mybir as mybir
import concourse.tile as tile
from concourse.bass_utils import run_bass_kernel_spmd

B = 1024
N = 16384
NCORES = 8
SPC = B // NCORES  # samples per core = 128
MIN_IDX = 364
MAX_IDX = 2276
DENOM = float(MAX_IDX - MIN_IDX - 3)  # 1909
C0, NC16 = 2, 16  # c in [2,18)
CHUNK = 8
NCH = SPC // CHUNK  # 16
NP64 = 64  # conjugate pairs

_IN_DT = mybir.dt.float16


def _build_weights():
    a = np.arange(128)
    d65 = np.arange(65)
    ang1 = 2.0 * np.pi * np.outer(a, d65) / 128.0
    cs = np.zeros((128, 128), np.float32)
    cs[:, :65] = np.cos(ang1) / 128.0
    cs[:, 65:] = -np.sin(ang1[:, 1:64]) / 128.0  # Yi_d, d=1..63

    b = np.arange(128)
    cg = np.arange(C0, C0 + NC16)
    dall = np.arange(128)
    k = 128 * cg[None, :] + dall[:, None]  # [d, c]
    ang2 = 2.0 * np.pi * b[None, :, None] * k[:, None, :] / float(N)  # [d,b,c]
    mr = np.cos(ang2) / 128.0
    mi = -np.sin(ang2) / 128.0
    oob = ((k < MIN_IDX) | (k >= MAX_IDX))[:, None, :]
    mr = np.where(oob, 0.0, mr)
    mi = np.where(oob, 0.0, mi)

    r1 = np.zeros((128, NP64, 64), np.float32)
    r2 = np.zeros((128, NP64, 64), np.float32)
    # pair 0 = (d=0, d=64), both have Yi == 0
    r1[:, 0, 0:16] = mr[0]
    r1[:, 0, 16:32] = mi[0]
    r2[:, 0, 32:48] = mr[64]
    r2[:, 0, 48:64] = mi[64]
    for p in range(1, NP64):
        dA, dB = p, 128 - p  # Yr_dB = Yr_dA, Yi_dB = -Yi_dA
        r1[:, p, 0:16] = mr[dA]
        r1[:, p, 16:32] = mi[dA]
        r1[:, p, 32:48] = mr[dB]
        r1[:, p, 48:64] = mi[dB]
        r2[:, p, 0:16] = -mi[dA]
        r2[:, p, 16:32] = mr[dA]
        r2[:, p, 32:48] = mi[dB]
        r2[:, p, 48:64] = -mr[dB]
    return (
        cs.astype(np.float16),
        r1.reshape(128, NP64 * 64).astype(np.float16),
        r2.reshape(128, NP64 * 64).astype(np.float16),
    )


def _ref_indices(t_hz):
    # exact argmin over f_i = 15*i/8192 with first-min tie-break
    u = t_hz.astype(np.float64) * 8192.0 / 15.0
    lo = np.floor(u).astype(np.int64)
    f = 15.0 / 8192.0
    d_lo = np.abs(lo * f - u * f)
    d_hi = np.abs((lo + 1) * f - u * f)
    return np.where(d_lo <= d_hi, lo, lo + 1)


def _strip_implied_waits(nc):
    """Drop sync waits that are transitively implied by other waits.

    The BIR/walrus backend allows only 1 sync wait on Matmult and DMACopy
    descriptors. Tile emits conservative wait sets (e.g. a PSUM WAW wait on
    PE plus a WAR wait on the drain engine, where the drain itself already
    waited on that PE value). A wait (S>=v) is droppable when another kept
    wait (S'>=v') has a producer whose happens-before closure already
    guarantees S>=v. Each DMA HW queue is modeled as its own FIFO engine.
    """
    from collections import defaultdict

    insts = [i for blk in nc.m.functions[0].blocks for i in blk.instructions]

    timeline = defaultdict(list)  # sem id -> [(cum_value, inst_index)]
    upd_sem = {}
    for idx, inst in enumerate(insts):
        si = inst.sync_info
        if not si:
            continue
        for u in si.on_update:
            if u.update_value is None:
                continue
            prev = timeline[u.id][-1][0] if timeline[u.id] else 0
            timeline[u.id].append((prev + u.update_value, idx))
            if "DMAHW" in (u.ant_name or "") or idx not in upd_sem:
                upd_sem[idx] = u.id

    def producer(sem_id, val):
        for cum, idx in timeline[sem_id]:
            if cum >= val:
                return idx
        return None

    def veng(idx):
        inst = insts[idx]
        if isinstance(inst, mybir.InstDMACopy) and idx in upd_sem:
            return ("q", upd_sem[idx])
        return ("e", str(inst.engine))

    # closure[idx]: sem id -> guaranteed min value at completion of insts[idx]
    closure = {}
    running = defaultdict(dict)  # veng -> sem floor dict
    for idx, inst in enumerate(insts):
        g = dict(running[veng(idx)])
        si = inst.sync_info
        if si:
            for w in si.on_wait:
                if w.wait_value is None:
                    continue
                g[w.id] = max(g.get(w.id, 0), w.wait_value)
                p = producer(w.id, w.wait_value)
                if p is not None and p < idx and p in closure:
                    for s, v in closure[p].items():
                        g[s] = max(g.get(s, 0), v)
            for u in si.on_update:
                if u.update_value is None:
                    continue
                cum = next(c for c, i2 in timeline[u.id] if i2 == idx)
                g[u.id] = max(g.get(u.id, 0), cum)
        closure[idx] = g
        running[veng(idx)] = g

    limits = {mybir.InstMatmult: 1, mybir.InstDMACopy: 1}
    for idx, inst in enumerate(insts):
        si = inst.sync_info
        if not si or len(si.on_wait) <= 1:
            continue
        kept = list(si.on_wait)
        changed = True
        while changed and len(kept) > 1:
            changed = False
            for w in kept:
                if w.wait_value is None:
                    continue
                implied = False
                for w2 in kept:
                    if w2 is w or w2.wait_value is None:
                        continue
                    p = producer(w2.id, w2.wait_value)
                    if (
                        p is not None
                        and p < idx
                        and closure.get(p, {}).get(w.id, 0) >= w.wait_value
                    ):
                        implied = True
                        break
                if implied:
                    kept.remove(w)
                    changed = True
                    break
        if len(kept) < len(si.on_wait):
            si.on_wait = kept
        lim = limits.get(type(inst))
        assert lim is None or len(kept) <= lim, (
            f"{type(inst).__name__} {inst.name} has {len(kept)} waits: "
            f"{[(w.ant_name, w.wait_value) for w in kept]}"
        )


def _build_program():
    nc = bass.Bass("TRN2", target_bir_lowering=False, debug=False, num_devices=NCORES)
    f32 = mybir.dt.float32

    x_d = nc.dram_tensor("x", [SPC, N], _IN_DT, kind="ExternalInput").ap()
    cs_d = nc.dram_tensor("cs", [128, 128], _IN_DT, kind="ExternalInput").ap()
    r1_d = nc.dram_tensor("r1", [128, NP64 * 64], _IN_DT, kind="ExternalInput").ap()
    r2_d = nc.dram_tensor("r2", [128, NP64 * 64], _IN_DT, kind="ExternalInput").ap()
    p1_d = nc.dram_tensor("p1", [SPC, NP64 * 32], f32, kind="ExternalOutput").ap()

    with tile.TileContext(nc) as tc:
        with (
            tc.tile_pool(name="const", bufs=1) as const,
            tc.tile_pool(name="xin", bufs=3) as xin,
            tc.tile_pool(name="big", bufs=1) as big,
            tc.tile_pool(name="ps", bufs=2, space="PSUM") as ps,
        ):
            cs_sb = const.tile([128, 128], _IN_DT)
            nc.sync.dma_start(out=cs_sb, in_=cs_d)
            r1_sb = const.tile([128, NP64, 64], _IN_DT)
            nc.sync.dma_start(out=r1_sb, in_=r1_d)
            r2_sb = const.tile([128, NP64, 64], _IN_DT)
            nc.sync.dma_start(out=r2_sb, in_=r2_d)

            y_sb = big.tile([128, SPC, 128], _IN_DT)  # [b, s, Yr0..64|Yi1..63]
            sq = big.tile([SPC, NP64, 64], f32)
            p1_sb = big.tile([SPC, NP64, 32], f32)

            def dummy_mm(out_cell, lhs_cell, rhs_cell):
                # N=1 matmuls fail the walrus ISA check; use [1,1]x[1,2]
                nc.tensor.matmul(
                    out=out_cell, lhsT=lhs_cell, rhs=rhs_cell, start=True, stop=True
                )

            # stage 1: per chunk of 8 samples, 8 matmuls into a 2-bank PSUM
            # tile, one big drain alternating ACT/DVE.
            yc0 = ps.tile([128, CHUNK, 128], f32, tag="ps")
            # warmup absorbers: PE waits once on each const DMA
            dummy_mm(yc0[0:1, 0, 0:2], cs_sb[0:1, 0:1], cs_sb[0:1, 0:2])
            dummy_mm(yc0[0:1, 0, 0:2], r1_sb[0:1, 0, 0:1], r1_sb[0:1, 0, 0:2])
            dummy_mm(yc0[0:1, 0, 0:2], r2_sb[0:1, 0, 0:1], r2_sb[0:1, 0, 0:2])

            for c in range(NCH):
                yc = yc0 if c == 0 else ps.tile([128, CHUNK, 128], f32, tag="ps")
                xt = xin.tile([128, CHUNK, 128], _IN_DT)  # [a, s, b]
                nc.sync.dma_start(
                    out=xt,
                    in_=x_d[c * CHUNK : (c + 1) * CHUNK, :].rearrange(
                        "s (a b) -> a s b", b=128
                    ),
                )
                if c >= 2:
                    # absorb the WAR wait on chunk c-2's drain engine
                    dummy_mm(yc[0:1, 0, 0:2], cs_sb[0:1, 0:1], cs_sb[0:1, 0:2])
                for si in range(CHUNK):
                    nc.tensor.matmul(
                        out=yc[:, si, :],
                        lhsT=xt[:, si, :],
                        rhs=cs_sb,
                        start=True,
                        stop=True,
                    )
                dst = y_sb[:, c * CHUNK : (c + 1) * CHUNK, :]
                if c % 2 == 0:
                    nc.scalar.copy(out=dst, in_=yc)
                else:
                    nc.vector.tensor_copy(out=dst, in_=yc)

            # stage 2: 8 groups x 8 conjugate pairs; per pair two matmuls
            # (stationary Yr_p then Yi_p) of 64 moving cols into PSUM.
            xp0 = ps.tile([SPC, 8, 64], f32, tag="ps")
            # absorb y_sb readiness: last ACT drain (chunk 14), last DVE (15)
            dummy_mm(
                xp0[0:1, 0, 0:2], y_sb[0:1, 14 * CHUNK, 0:1], y_sb[0:1, 14 * CHUNK, 0:2]
            )
            dummy_mm(
                xp0[0:1, 0, 0:2], y_sb[0:1, 15 * CHUNK, 0:1], y_sb[0:1, 15 * CHUNK, 0:2]
            )
            for g in range(8):
                xp = xp0 if g == 0 else ps.tile([SPC, 8, 64], f32, tag="ps")
                for pi in range(8):
                    p = g * 8 + pi
                    nc.tensor.matmul(
                        out=xp[:, pi, :],
                        lhsT=y_sb[:, :, p],
                        rhs=r1_sb[:, p, :],
                        start=True,
                        stop=False,
                    )
                    nc.tensor.matmul(
                        out=xp[:, pi, :],
                        lhsT=y_sb[:, :, 64 + p],
                        rhs=r2_sb[:, p, :],
                        start=False,
                        stop=True,
                    )
                nc.scalar.activation(
                    out=sq[:, g * 8 : (g + 1) * 8, :],
                    in_=xp,
                    func=mybir.ActivationFunctionType.Square,
                )

            # P1[s, p, 0:16] = Xr_dA^2 + Xi_dA^2 ; [16:32] for dB
            nc.vector.tensor_add(
                out=p1_sb[:, :, 0:16], in0=sq[:, :, 0:16], in1=sq[:, :, 16:32]
            )
            nc.vector.tensor_add(
                out=p1_sb[:, :, 16:32], in0=sq[:, :, 32:48], in1=sq[:, :, 48:64]
            )
            nc.sync.dma_start(out=p1_d, in_=p1_sb)

    _strip_implied_waits(nc)
    return nc


_CACHE = {}


def _get_program():
    if "nc" not in _CACHE:
        _CACHE["nc"] = _build_program()
        _CACHE["w"] = _build_weights()
    return _CACHE["nc"], _CACHE["w"]


def _run(outputs, targets, trace=False):
    nc, (cs, r1, r2) = _get_program()
    xh = np.ascontiguousarray(np.asarray(outputs).astype(np.float16))
    targets = np.asarray(targets, dtype=np.float32)

    in_maps = [
        {"x": xh[i * SPC : (i + 1) * SPC], "cs": cs, "r1": r1, "r2": r2}
        for i in range(NCORES)
    ]
    res = run_bass_kernel_spmd(nc, in_maps, list(range(NCORES)), trace=trace)
    p1 = np.concatenate([res.results[i]["p1"] for i in range(NCORES)], axis=0)
    p1 = p1.reshape(B, NP64, 2, NC16).astype(np.float64)

    t_hz = targets[:, 0].astype(np.float64) / 60.0
    ref = _ref_indices(t_hz)

    def pval(k):
        d = k % 128
        c = k // 128 - C0
        p = np.where(d % 64 == 0, 0, np.where(d < 64, d, 128 - d))
        r = np.where(d == 0, 0, np.where(d >= 64, 1, 0))
        return p1[np.arange(B), p, r, c]

    band = p1.sum(axis=(1, 2, 3))
    excl = pval(ref - 1) + pval(ref) + pval(ref + 1)
    pulse = pval(ref)
    other = (band - excl) / DENOM
    snr = 10.0 * np.log10(pulse / other)
    loss = -np.mean(snr)
    return np.float32(loss), res.exec_time_ns


def kernel(**inputs):
    loss, _ = _run(inputs["outputs"], inputs["targets"], trace=False)
    return np.asarray(loss, dtype=np.float32)


# revision 20
# speedup vs baseline: 2.2166x; 1.1407x over previous
"""NegSNR loss on TRN2: two-stage rfft (16384 = 128x128) as fp16 matmuls.

Per core (128 samples): stage 1 computes Y[b,d] = sum_a x[128a+b] W128^{ad}
for d in [0,64] only (x real => Y_{128-d} = conj(Y_d)): 65 cos cols + 63 sin
cols = 128 columns. Stage 2 folds the twiddle W_N^{bd} and the outer DFT
into per-d constant matrices; conjugate pairs (d, 128-d) share the same
stationary Y columns, so each of the 64 pairs needs just two matmuls of 64
moving columns: X over in-band bins k in [364, 2276) (c in [2,18)), with
out-of-band columns zeroed. P1 = Xr^2 + Xi^2 is returned per core; the
final gather/log/mean runs on host.

Sync constraint: the BIR/walrus path cannot split >1 semaphore wait on a
Matmult (fused ldweights) or DMACopy. Tiny [1x1]x[1x2] "absorber" matmuls
soak up each cross-engine dependency so every real matmul carries at most
one wait, and _strip_implied_waits drops waits already implied transitively.
"""

import numpy as np

import concourse.bass as bass
import concourse.mybir as mybir
import concourse.tile as tile
from concourse.bass_utils import run_bass_kernel_spmd

B = 1024
N = 16384
NCORES = 8
SPC = B // NCORES  # samples per core = 128
MIN_IDX = 364
MAX_IDX = 2276
DENOM = float(MAX_IDX - MIN_IDX - 3)  # 1909
C0, NC16 = 2, 16  # c in [2,18)
CHUNK = 8
NCH = SPC // CHUNK  # 16
NP64 = 64  # conjugate pairs

_IN_DT = mybir.dt.float16


def _build_weights():
    a = np.arange(128)
    d65 = np.arange(65)
    ang1 = 2.0 * np.pi * np.outer(a, d65) / 128.0
    cs = np.zeros((128, 128), np.float32)
    cs[:, :65] = np.cos(ang1) / 128.0
    cs[:, 65:] = -np.sin(ang1[:, 1:64]) / 128.0  # Yi_d, d=1..63

    b = np.arange(128)
    cg = np.arange(C0, C0 + NC16)
    dall = np.arange(128)
    k = 128 * cg[None, :] + dall[:, None]  # [d, c]
    ang2 = 2.0 * np.pi * b[None, :, None] * k[:, None, :] / float(N)  # [d,b,c]
    mr = np.cos(ang2) / 128.0
    mi = -np.sin(ang2) / 128.0
    oob = ((k < MIN_IDX) | (k >= MAX_IDX))[:, None, :]
    mr = np.where(oob, 0.0, mr)
    mi = np.where(oob, 0.0, mi)

    r1 = np.zeros((128, NP64, 64), np.float32)
    r2 = np.zeros((128, NP64, 64), np.float32)
    # pair 0 = (d=0, d=64), both have Yi == 0
    r1[:, 0, 0:16] = mr[0]
    r1[:, 0, 16:32] = mi[0]
    r2[:, 0, 32:48] = mr[64]
    r2[:, 0, 48:64] = mi[64]
    for p in range(1, NP64):
        dA, dB = p, 128 - p  # Yr_dB = Yr_dA, Yi_dB = -Yi_dA
        r1[:, p, 0:16] = mr[dA]
        r1[:, p, 16:32] = mi[dA]
        r1[:, p, 32:48] = mr[dB]
        r1[:, p, 48:64] = mi[dB]
        r2[:, p, 0:16] = -mi[dA]
        r2[:, p, 16:32] = mr[dA]
        r2[:, p, 32:48] = mi[dB]
        r2[:, p, 48:64] = -mr[dB]
    return (
        cs.astype(np.float16),
        r1.reshape(128, NP64 * 64).astype(np.float16),
        r2.reshape(128, NP64 * 64).astype(np.float16),
    )


def _ref_indices(t_hz):
    # exact argmin over f_i = 15*i/8192 with first-min tie-break
    u = t_hz.astype(np.float64) * 8192.0 / 15.0
    lo = np.floor(u).astype(np.int64)
    f = 15.0 / 8192.0
    d_lo = np.abs(lo * f - u * f)
    d_hi = np.abs((lo + 1) * f - u * f)
    return np.where(d_lo <= d_hi, lo, lo + 1)


def _strip_implied_waits(nc):
    """Drop sync waits that are transitively implied by other waits.

    The BIR/walrus backend allows only 1 sync wait on Matmult and DMACopy
    descriptors. Tile emits conservative wait sets (e.g. a PSUM WAW wait on
    PE plus a WAR wait on the drain engine, where the drain itself already
    waited on that PE value). A wait (S>=v) is droppable when another kept
    wait (S'>=v') has a producer whose happens-before closure already
    guarantees S>=v. Each DMA HW queue is modeled as its own FIFO engine.
    """
    from collections import defaultdict

    insts = [i for blk in nc.m.functions[0].blocks for i in blk.instructions]

    timeline = defaultdict(list)  # sem id -> [(cum_value, inst_index)]
    upd_sem = {}
    for idx, inst in enumerate(insts):
        si = inst.sync_info
        if not si:
            continue
        for u in si.on_update:
            if u.update_value is None:
                continue
            prev = timeline[u.id][-1][0] if timeline[u.id] else 0
            timeline[u.id].append((prev + u.update_value, idx))
            if "DMAHW" in (u.ant_name or "") or idx not in upd_sem:
                upd_sem[idx] = u.id

    def producer(sem_id, val):
        for cum, idx in timeline[sem_id]:
            if cum >= val:
                return idx
        return None

    def veng(idx):
        inst = insts[idx]
        if isinstance(inst, mybir.InstDMACopy) and idx in upd_sem:
            return ("q", upd_sem[idx])
        return ("e", str(inst.engine))

    # closure[idx]: sem id -> guaranteed min value at completion of insts[idx]
    # prev_floor[idx]: guarantees already held before idx starts, via in-order
    # execution on its own engine / DMA queue
    closure = {}
    prev_floor = {}
    running = defaultdict(dict)  # veng -> sem floor dict
    for idx, inst in enumerate(insts):
        g = dict(running[veng(idx)])
        prev_floor[idx] = dict(g)
        si = inst.sync_info
        if si:
            for w in si.on_wait:
                if w.wait_value is None:
                    continue
                g[w.id] = max(g.get(w.id, 0), w.wait_value)
                p = producer(w.id, w.wait_value)
                if p is not None and p < idx and p in closure:
                    for s, v in closure[p].items():
                        g[s] = max(g.get(s, 0), v)
            for u in si.on_update:
                if u.update_value is None:
                    continue
                cum = next(c for c, i2 in timeline[u.id] if i2 == idx)
                g[u.id] = max(g.get(u.id, 0), cum)
        closure[idx] = g
        running[veng(idx)] = g

    limits = {mybir.InstMatmult: 1, mybir.InstDMACopy: 1}
    for idx, inst in enumerate(insts):
        si = inst.sync_info
        if not si or len(si.on_wait) <= 1:
            continue
        kept = [
            w
            for w in si.on_wait
            if w.wait_value is None
            or prev_floor[idx].get(w.id, 0) < w.wait_value
        ]
        changed = True
        while changed and len(kept) > 1:
            changed = False
            for w in kept:
                if w.wait_value is None:
                    continue
                implied = False
                for w2 in kept:
                    if w2 is w or w2.wait_value is None:
                        continue
                    p = producer(w2.id, w2.wait_value)
                    if (
                        p is not None
                        and p < idx
                        and closure.get(p, {}).get(w.id, 0) >= w.wait_value
                    ):
                        implied = True
                        break
                if implied:
                    kept.remove(w)
                    changed = True
                    break
        if len(kept) < len(si.on_wait):
            si.on_wait = kept
        lim = limits.get(type(inst))
        assert lim is None or len(kept) <= lim, (
            f"{type(inst).__name__} {inst.name} has {len(kept)} waits: "
            f"{[(w.ant_name, w.wait_value) for w in kept]}"
        )

    # Drain (TPB_CTRL) also allows only 1 wait: split multi-wait drains
    # (e.g. the final flush of all 8 DMA HW queues) into a chain of
    # single-wait drains on the same engine.
    for blk in nc.m.functions[0].blocks:
        i = 0
        while i < len(blk.instructions):
            inst = blk.instructions[i]
            si = inst.sync_info
            if isinstance(inst, mybir.InstDrain) and si and len(si.on_wait) > 1:
                extra = si.on_wait[:-1]
                si.on_wait = si.on_wait[-1:]
                for w in extra:
                    d = mybir.InstDrain(
                        name=nc.get_next_instruction_name(), ins=[], outs=[],
                        bass_is_fusable=False,
                    )
                    d.engine = inst.engine
                    d.sync_info = mybir.SyncInfo(on_wait=[w], on_update=[])
                    blk.instructions.insert(i, d)
                    i += 1
            i += 1


def _build_program():
    nc = bass.Bass("TRN2", target_bir_lowering=False, debug=False, num_devices=NCORES)
    f32 = mybir.dt.float32

    x_d = nc.dram_tensor("x", [SPC, N], _IN_DT, kind="ExternalInput").ap()
    cs_d = nc.dram_tensor("cs", [128, 128], _IN_DT, kind="ExternalInput").ap()
    r1_d = nc.dram_tensor("r1", [128, NP64 * 64], _IN_DT, kind="ExternalInput").ap()
    r2_d = nc.dram_tensor("r2", [128, NP64 * 64], _IN_DT, kind="ExternalInput").ap()
    p1_d = nc.dram_tensor("p1", [SPC, NP64, 32], f32, kind="ExternalOutput").ap()

    with tile.TileContext(nc) as tc:
        with (
            tc.tile_pool(name="const", bufs=1) as const,
            tc.tile_pool(name="xin", bufs=4) as xin,
            tc.tile_pool(name="big", bufs=1) as big,
            tc.tile_pool(name="ps", bufs=4, space="PSUM") as ps,
        ):
            cs_sb = const.tile([128, 128], _IN_DT)
            nc.sync.dma_start(out=cs_sb, in_=cs_d)
            # r1/r2 ride the ACT HWDGE queue so they don't delay x chunks
            # on the SP queue; they are only needed at stage 2.
            r1_sb = const.tile([128, NP64, 64], _IN_DT)
            nc.scalar.dma_start(out=r1_sb, in_=r1_d)
            r2_sb = const.tile([128, NP64, 64], _IN_DT)
            nc.scalar.dma_start(out=r2_sb, in_=r2_d)

            y_sb = big.tile([128, SPC, 128], _IN_DT)  # [b, s, Yr0..64|Yi1..63]
            sq = big.tile([SPC, NP64, 64], f32)
            p1_sb = big.tile([SPC, NP64, 32], f32)

            def dummy_mm(out_cell, lhs_cell, rhs_cell):
                # N=1 matmuls fail the walrus ISA check; use [1,1]x[1,2]
                nc.tensor.matmul(
                    out=out_cell, lhsT=lhs_cell, rhs=rhs_cell, start=True, stop=True
                )

            # stage 1: per chunk of 8 samples, 8 matmuls into a 2-bank PSUM
            # tile, one big drain alternating ACT/DVE.
            yc0 = ps.tile([128, CHUNK, 128], f32, tag="ps")
            # warmup absorber: PE waits once on the cs DMA
            dummy_mm(yc0[0:1, 0, 0:2], cs_sb[0:1, 0:1], cs_sb[0:1, 0:2])

            for c in range(NCH):
                yc = yc0 if c == 0 else ps.tile([128, CHUNK, 128], f32, tag="ps")
                xt = xin.tile([128, CHUNK, 128], _IN_DT)  # [a, s, b]
                nc.sync.dma_start(
                    out=xt,
                    in_=x_d[c * CHUNK : (c + 1) * CHUNK, :].rearrange(
                        "s (a b) -> a s b", b=128
                    ),
                )
                if c >= 4:
                    # absorb the WAR wait on chunk c-4's drain engine
                    dummy_mm(yc[0:1, 0, 0:2], cs_sb[0:1, 0:1], cs_sb[0:1, 0:2])
                for si in range(CHUNK):
                    nc.tensor.matmul(
                        out=yc[:, si, :],
                        lhsT=xt[:, si, :],
                        rhs=cs_sb,
                        start=True,
                        stop=True,
                    )
                dst = y_sb[:, c * CHUNK : (c + 1) * CHUNK, :]
                if c % 2 == 0:
                    nc.scalar.copy(out=dst, in_=yc)
                else:
                    nc.vector.tensor_copy(out=dst, in_=yc)

            # stage 2: 8 groups x 8 conjugate pairs; per pair two matmuls
            # (stationary Yr_p then Yi_p) of 64 moving cols into PSUM.
            xp0 = ps.tile([SPC, 8, 64], f32, tag="ps")
            # absorb drains of chunks 12..15 (covers xp slot WARs and y_sb
            # readiness), then the r1/r2 const DMAs.
            for c in (12, 13, 14, 15):
                dummy_mm(
                    xp0[0:1, 0, 0:2],
                    y_sb[0:1, c * CHUNK, 0:1],
                    y_sb[0:1, c * CHUNK, 0:2],
                )
            dummy_mm(xp0[0:1, 0, 0:2], r1_sb[0:1, 0, 0:1], r1_sb[0:1, 0, 0:2])
            dummy_mm(xp0[0:1, 0, 0:2], r2_sb[0:1, 0, 0:1], r2_sb[0:1, 0, 0:2])
            for g in range(8):
                xp = xp0 if g == 0 else ps.tile([SPC, 8, 64], f32, tag="ps")
                for pi in range(8):
                    p = g * 8 + pi
                    nc.tensor.matmul(
                        out=xp[:, pi, :],
                        lhsT=y_sb[:, :, p],
                        rhs=r1_sb[:, p, :],
                        start=True,
                        stop=False,
                    )
                    nc.tensor.matmul(
                        out=xp[:, pi, :],
                        lhsT=y_sb[:, :, 64 + p],
                        rhs=r2_sb[:, p, :],
                        start=False,
                        stop=True,
                    )
                gs = slice(g * 8, (g + 1) * 8)
                nc.scalar.activation(
                    out=sq[:, gs, :],
                    in_=xp,
                    func=mybir.ActivationFunctionType.Square,
                )
                # P1[s, p, 0:16] = Xr_dA^2 + Xi_dA^2 ; [16:32] for dB
                nc.vector.tensor_add(
                    out=p1_sb[:, gs, 0:16], in0=sq[:, gs, 0:16], in1=sq[:, gs, 16:32]
                )
                nc.vector.tensor_add(
                    out=p1_sb[:, gs, 16:32], in0=sq[:, gs, 32:48], in1=sq[:, gs, 48:64]
                )
                nc.sync.dma_start(out=p1_d[:, gs, :], in_=p1_sb[:, gs, :])

    _strip_implied_waits(nc)
    return nc


_CACHE = {}


def _get_program():
    if "nc" not in _CACHE:
        _CACHE["nc"] = _build_program()
        _CACHE["w"] = _build_weights()
    return _CACHE["nc"], _CACHE["w"]


def _run(outputs, targets, trace=False):
    nc, (cs, r1, r2) = _get_program()
    xh = np.ascontiguousarray(np.asarray(outputs).astype(np.float16))
    targets = np.asarray(targets, dtype=np.float32)

    in_maps = [
        {"x": xh[i * SPC : (i + 1) * SPC], "cs": cs, "r1": r1, "r2": r2}
        for i in range(NCORES)
    ]
    res = run_bass_kernel_spmd(nc, in_maps, list(range(NCORES)), trace=trace)
    p1 = np.concatenate([res.results[i]["p1"] for i in range(NCORES)], axis=0)
    p1 = p1.reshape(B, NP64, 2, NC16).astype(np.float64)

    t_hz = targets[:, 0].astype(np.float64) / 60.0
    ref = _ref_indices(t_hz)

    def pval(k):
        d = k % 128
        c = k // 128 - C0
        p = np.where(d % 64 == 0, 0, np.where(d < 64, d, 128 - d))
        r = np.where(d == 0, 0, np.where(d >= 64, 1, 0))
        return p1[np.arange(B), p, r, c]

    band = p1.sum(axis=(1, 2, 3))
    excl = pval(ref - 1) + pval(ref) + pval(ref + 1)
    pulse = pval(ref)
    other = (band - excl) / DENOM
    snr = 10.0 * np.log10(pulse / other)
    loss = -np.mean(snr)
    return np.float32(loss), res.exec_time_ns


def kernel(**inputs):
    loss, _ = _run(inputs["outputs"], inputs["targets"], trace=False)
    return np.asarray(loss, dtype=np.float32)
